# revision 11
# baseline (speedup 1.0000x reference)
"""Trainium2 Bass kernel for nn_InpaintContextAttentionUnit.

Per-sample computation (B=8 samples -> 1 per NeuronCore):
  fm [512,512,16] -> avgpool(64x2) -> pooled [8,256,16]
  -> two masked 3x3 convs (middle row / middle col of kernel zeroed) + bias + relu
  -> bilinear upsample back to [512,512,16] (separable; half-pixel centers, edge clamp)
  -> out [512,512,48] = concat(fm, fm - row_up, fm - col_up)

Design (v2 — single fm read, contiguous DVE paths, DMA-overlap-first):
  - fm is loaded from HBM exactly once (4x 4 MiB fp32 HWDGE loads); ACT casts each
    tile into a persistent bf16 copy used by pooling AND by the pass-B subtract /
    passthrough (bf16 roundtrip error ~2^-9 rel, well under the 2e-2 gate)
  - pooling: PE matmul with a [128,2] block-mean matrix; rhs kept (xp, c)-inner-
    contiguous (strided rhs costs ~5 cyc/col on PE); contiguous CAST to bf16
    stage, SBUF-hop to an assembled ncw [8n, (xp c)]; the c<->w free transpose
    runs ONCE on 8 partitions (split DVE/ACT halves) instead of 4x on 2
    partitions (v1: 18us each)
  - conv: per (branch, n-pair chunk): zero-init matmul + ~6 accumulating
    [16c,16f]x[16c,<=512] matmuls in PSUM; relu+bias on ACT; per-branch DRAM
    bounce so branch-0 W-upsample (DVE) overlaps branch-1 conv (PE)
  - W-upsample (x2): 2 scalar_tensor_tensor ops per branch computing
    pad[k]*1/3 + pad[k+1] (the 0.75 factor is folded into the host hup matrix,
    exact in bf16), written (f, x)-major (x-inner iteration is the fast STT path)
  - H-upsample (x64): PE matmuls rw[8n, (f, x)] with host-built 0.75*HUp matrix
  - combine: DVE subtract (bf16 fm - psum) + ACT copy into interleaved
    [y, x, 48ch] staging tiles (bufs=4), contiguous 3 MiB DMAs out
All constant matrices are precomputed on host and passed as extra inputs.
"""

import numpy as np
import ml_dtypes

H, W, C, F = 512, 512, 16, 16
NPOOL = 8
WP = W // 2  # 256
CH_OUT = 3 * C  # 48

_cache = {}


def _host_consts(kernel, bias):
    """Build host-side constant matrices (bf16 for the PE-side constants)."""
    bf = ml_dtypes.bfloat16
    # pooling weights: [128, 2], 1/128 (exact in bf16) where row block matches
    poolw = np.zeros((128, 2), np.float32)
    poolw[:64, 0] = 1.0 / 128.0
    poolw[64:, 1] = 1.0 / 128.0
    # H-upsample matrix: hup[n, y] = weight of pooled row n for output row y,
    # scaled by 0.75 (the W-upsample major tap; k/64*0.75 = 3k/256 exact in bf16)
    hup = np.zeros((NPOOL, H), np.float32)
    scale = H // NPOOL
    for y in range(H):
        yf = (y + 0.5) / scale - 0.5
        i0 = int(np.floor(yf))
        w = yf - i0
        hup[min(max(i0, 0), NPOOL - 1), y] += 1.0 - w
        hup[min(max(i0 + 1, 0), NPOOL - 1), y] += w
    hup *= 0.75
    hup2 = np.zeros((40, H), np.float32)
    hup2[0:8] = hup
    hup2[32:40] = hup  # col-branch copy at base partition 32; rows 8-15 stay zero
    # conv taps: branch 0 (row conv): K[dn+1, dwp+1]; branch 1 (col): K[dwp+1, dn+1]
    taps0 = [(dn, dwp) for dn in (-1, 1) for dwp in (-1, 0, 1)]
    taps1 = [(dn, dwp) for dwp in (-1, 1) for dn in (-1, 0, 1)]
    kt = np.zeros((16, 13 * 16), np.float32)  # [c, tap*16+f]; slot 12 = zeros
    for i, (dn, dwp) in enumerate(taps0):
        kt[:, i * 16:(i + 1) * 16] = kernel[dn + 1, dwp + 1]
    for i, (dn, dwp) in enumerate(taps1):
        kt[:, (6 + i) * 16:(7 + i) * 16] = kernel[dwp + 1, dn + 1]
    bias2 = np.ascontiguousarray(bias.reshape(16, 1)).astype(np.float32)
    return (poolw.astype(bf), hup2.astype(bf), kt.astype(bf), bias2, taps0, taps1)


def _build_program(compile=True):
    import concourse.bass as bass
    import concourse.bacc as bacc
    import concourse.mybir as mybir
    import concourse.tile as tile

    dt = mybir.dt.float32
    db = mybir.dt.bfloat16
    nc = bacc.Bacc()

    fm_d = nc.declare_dram_parameter("feature_map", [H, W, C], dt, isOutput=False)
    poolw_d = nc.declare_dram_parameter("poolw", [128, 2], db, isOutput=False)
    hup_d = nc.declare_dram_parameter("hup", [40, H], db, isOutput=False)
    ktaps_d = nc.declare_dram_parameter("ktaps", [16, 208], db, isOutput=False)
    bias_d = nc.declare_dram_parameter("bias2", [16, 1], dt, isOutput=False)
    out_d = nc.declare_dram_parameter("out", [H, W, CH_OUT], dt, isOutput=True)

    taps0 = [(dn, dwp) for dn in (-1, 1) for dwp in (-1, 0, 1)]
    taps1 = [(dn, dwp) for dwp in (-1, 1) for dn in (-1, 0, 1)]
    taps_by_branch = [taps0, taps1]

    with tile.TileContext(nc) as tc:
        with (
            tc.tile_pool(name="consts", bufs=1) as cpool,
            tc.tile_pool(name="persist", bufs=1) as ppool,
        ):
            # const tiles (DMAs are emitted after the first fm load below so
            # the big load heads the queue)
            poolw_t = cpool.tile([128, 2], db)
            hup_t = cpool.tile([40, H], db)
            ktaps_t = cpool.tile([16, 208], db)
            bias_t = cpool.tile([16, 1], dt)

            # persistent bf16 fm copy: [128, (4 t, 512 x, 16 c)]
            fmb_t = ppool.tile([128, 4 * W * C], db)
            # rw [40, (16 f, 512 x)] bf16: partitions 0-7 row-branch, 32-39 col
            rw_t = ppool.tile([40, 16 * W], db)

            # ================= PASS A: pooling + conv + W-upsample =================
            with (
                tc.tile_pool(name="passA", bufs=1) as apool,
                tc.tile_pool(name="dram", bufs=1, space="DRAM") as dpool,
            ):
                # pooled DRAM bounce: [8 n, (16 c, 258 wp)] bf16 with zero wp-halo
                ncw_dram = dpool.tile([NPOOL, 16 * 258], db)
                nd3 = ncw_dram[:].rearrange("n (c w) -> n c w", w=258)
                zsrc = hup_d[8:16, 0:16]  # [8, 16] zeros

                # pooling rhs view of fmb: (t, xp, par, c) — c-inner contiguous
                fmr = fmb_t[:].rearrange(
                    "p (t xp par c) -> p t xp par c", t=4, par=2, c=16)

                # pooled_T [16 c, (8 n, 258 wp)], filled per-tile (n-direction
                # zero padding handled by clipped matmul n-ranges)
                tpad_t = apool.tile([16, NPOOL * 258], db)
                tpad3 = tpad_t[:].rearrange("p (n w) -> p n w", w=258)
                ncwd3 = ncw_dram[:].rearrange("n (c w) -> c n w", w=258)

                conv_t = apool.tile([16, 2 * NPOOL * WP], db)
                conv_dram = dpool.tile([16, 2 * NPOOL * WP], db)
                cd4 = conv_dram[:].rearrange("f (b n w) -> b n f w", b=2, n=NPOOL)
                rop_t = apool.tile([40, 16 * 258], db)
                rop3 = rop_t[:].rearrange("p (f w) -> p f w", w=258)
                rwv = rw_t[:].rearrange("p (f xp par) -> p f par xp", par=2, xp=WP)

                def conv_chunk(psC, b, ch):
                    # n-pair chunk: n in {2ch, 2ch+1}
                    n0 = 2 * ch
                    ps = psC.tile([16, 2 * WP], dt, tag="conv", bufs=4)
                    # zero-init whole chunk (ktaps slot 12 = zeros)
                    nc.tensor.matmul(
                        ps[:], ktaps_t[:, 192:208],
                        tpad3[:, n0:n0 + 2, 1:257],
                        start=True, stop=False, skip_group_check=True,
                    )
                    pieces = []
                    for i, (dn, dwp) in enumerate(taps_by_branch[b]):
                        nlo = max(n0, -dn)
                        nhi = min(n0 + 2, NPOOL - dn)
                        if nhi <= nlo:
                            continue
                        pieces.append((b * 6 + i, dn, dwp, nlo, nhi))
                    for k, (sl, dn, dwp, nlo, nhi) in enumerate(pieces):
                        nc.tensor.matmul(
                            ps[:, (nlo - n0) * WP:(nhi - n0) * WP],
                            ktaps_t[:, sl * 16:(sl + 1) * 16],
                            tpad3[:, nlo + dn:nhi + dn, 1 + dwp:257 + dwp],
                            start=False, stop=(k == len(pieces) - 1),
                            skip_group_check=True,
                        )
                    nc.scalar.activation(
                        out=conv_t[:, (b * NPOOL + n0) * WP:
                                   (b * NPOOL + n0 + 2) * WP],
                        in_=ps[:],
                        func=mybir.ActivationFunctionType.Relu,
                        bias=bias_t[:, 0:1],
                    )

                def branch_tail(b):
                    # bounce [16 f, (n, wp)] -> [(b,n) parts, (f, wp)] + W-up
                    nc.sync.dma_start(
                        out=conv_dram[:, b * NPOOL * WP:(b + 1) * NPOOL * WP],
                        in_=conv_t[:, b * NPOOL * WP:(b + 1) * NPOOL * WP])
                    pg = 32 * b  # partition base: row->0, col->32
                    nc.sync.dma_start(out=rop3[pg:pg + 8, :, 1:257], in_=cd4[b])
                    # edge replicate (W clamp)
                    nc.vector.tensor_copy(
                        rop3[pg:pg + 8, :, 0:1], rop3[pg:pg + 8, :, 1:2])
                    nc.vector.tensor_copy(
                        rop3[pg:pg + 8, :, 257:258], rop3[pg:pg + 8, :, 256:257])
                    # W-upsample into (f, x)-major rw; 0.75 folded into hup:
                    #   rw[2k]   = pad[k]/3   + pad[k+1]
                    #   rw[2k+1] = pad[k+2]/3 + pad[k+1]
                    third = 1.0 / 3.0
                    nc.vector.scalar_tensor_tensor(
                        out=rwv[pg:pg + 8, :, 0, :],
                        in0=rop3[pg:pg + 8, :, 0:256],
                        scalar=third,
                        in1=rop3[pg:pg + 8, :, 1:257],
                        op0=mybir.AluOpType.mult,
                        op1=mybir.AluOpType.add,
                    )
                    nc.vector.scalar_tensor_tensor(
                        out=rwv[pg:pg + 8, :, 1, :],
                        in0=rop3[pg:pg + 8, :, 2:258],
                        scalar=third,
                        in1=rop3[pg:pg + 8, :, 1:257],
                        op0=mybir.AluOpType.mult,
                        op1=mybir.AluOpType.add,
                    )

                # conv chunk ch needs pooled n in [2ch-1, 2ch+2] -> tpad tiles
                # up to ch+1; chunks 0/1 run under the tile-2/3 loads
                chunks_at = {1: [0], 2: [1], 3: [2, 3]}

                with (
                    tc.tile_pool(name="psA", bufs=1, space="PSUM") as psA,
                    tc.tile_pool(name="psConv", bufs=1, space="PSUM") as psC,
                ):
                    for t in range(4):
                        fmf = apool.tile([128, W * C], dt, tag="fmf", bufs=2)
                        fmf3 = fmf[:].rearrange("p (x c) -> p x c", c=C)
                        nc.sync.dma_start(out=fmf3, in_=fm_d[128 * t:128 * (t + 1)])
                        if t == 0:
                            # consts load behind the first big fm load
                            nc.sync.dma_start(out=poolw_t[:], in_=poolw_d[:])
                            nc.sync.dma_start(out=hup_t[:], in_=hup_d[:])
                            nc.sync.dma_start(out=ktaps_t[:], in_=ktaps_d[:])
                            nc.sync.dma_start(out=bias_t[:], in_=bias_d[:])
                            nc.sync.dma_start(out=nd3[:, :, 0:1], in_=zsrc)
                            nc.sync.dma_start(out=nd3[:, :, 257:258], in_=zsrc)
                        # bf16 cast split across ACT and DVE halves (GpSimd
                        # tensor ops measured ~4x slower than nominal)
                        half = W * C // 2
                        nc.scalar.activation(
                            out=fmb_t[:, t * W * C:t * W * C + half],
                            in_=fmf[:, 0:half],
                            func=mybir.ActivationFunctionType.Copy)
                        nc.vector.tensor_copy(
                            fmb_t[:, t * W * C + half:(t + 1) * W * C],
                            fmf[:, half:])
                        # H-pool (y->n) + W-pair add via PSUM accumulation;
                        # (xp, c)-major psum in 2-bank quarters (conv needs the
                        # other 4 banks concurrently)
                        stage = apool.tile([2, WP * 16], db, tag="stage", bufs=2)
                        for qt in range(4):
                            ps = psA.tile([2, 1024], dt, tag="pool", bufs=2)
                            for jj in range(2):
                                j = 2 * qt + jj
                                for par in range(2):
                                    nc.tensor.matmul(
                                        ps[:, 512 * jj:512 * (jj + 1)], poolw_t[:],
                                        fmr[:, t, 32 * j:32 * (j + 1), par, :],
                                        start=(par == 0), stop=(par == 1),
                                    )
                            # contiguous f32->bf16 copy, alternating DVE/ACT
                            dst = stage[:, 1024 * qt:1024 * (qt + 1)]
                            if qt % 2 == 0:
                                nc.vector.tensor_copy(dst, ps[:])
                            else:
                                nc.scalar.activation(
                                    out=dst, in_=ps[:],
                                    func=mybir.ActivationFunctionType.Copy)
                        # free-dim transpose (xp, c) -> (c, w), split DVE/ACT
                        stageT = apool.tile([2, WP * 16], db, tag="stageT", bufs=2)
                        st_cx = stage[:].rearrange("p (x c) -> p c x", c=16)
                        stT3 = stageT[:].rearrange("p (c x) -> p c x", x=WP)
                        nc.vector.tensor_copy(stT3[:, 0:8, :], st_cx[:, 0:8, :])
                        nc.scalar.activation(
                            out=stT3[:, 8:16, :], in_=st_cx[:, 8:16, :],
                            func=mybir.ActivationFunctionType.Copy)
                        # bounce to DRAM and read back the c-on-partitions slice
                        nc.sync.dma_start(
                            out=nd3[2 * t:2 * t + 2, :, 1:257], in_=stT3)
                        nc.sync.dma_start(
                            out=tpad3[:, 2 * t:2 * t + 2, :],
                            in_=ncwd3[:, 2 * t:2 * t + 2, :])
                        for ch in chunks_at.get(t, []):
                            if ch < 3:
                                conv_chunk(psC, 0, ch)
                                conv_chunk(psC, 1, ch)
                            else:
                                conv_chunk(psC, 0, ch)
                                branch_tail(0)
                                conv_chunk(psC, 1, ch)
                                branch_tail(1)

            # ================= PASS B: H-upsample + combine + store =================
            with (
                tc.tile_pool(name="passB", bufs=1) as bpool,
                tc.tile_pool(name="psB", bufs=2, space="PSUM") as psB,
            ):
                fmb4 = fmb_t[:].rearrange("p (t x c) -> p t x c", t=4, c=16)
                rwx = rw_t[:].rearrange("p (f x) -> p f x", x=W)
                for t in range(4):
                    for q in range(4):
                        outq = bpool.tile([128, 128 * CH_OUT], dt, tag="outq", bufs=4)
                        outq3 = outq[:].rearrange("p (x ch) -> p x ch", ch=CH_OUT)
                        fmq = fmb4[:, t, 128 * q:128 * (q + 1), :]
                        nc.scalar.activation(
                            out=outq3[:, :, 0:16], in_=fmq,
                            func=mybir.ActivationFunctionType.Copy,
                        )
                        for b in range(2):
                            pg = 32 * b
                            lhsT = hup_t[pg:pg + 8, 128 * t:128 * (t + 1)]  # [8,128]
                            ps = psB.tile([128, 128 * 16], dt, tag="up")
                            # ps is (f, x-slice)-major: [128, (4f, 128x)] per bank
                            for i in range(4):  # 4-f chunks: 512 f32 = 1 bank
                                nc.tensor.matmul(
                                    ps[:, 512 * i:512 * (i + 1)],
                                    lhsT,
                                    rwx[pg:pg + 8, 4 * i:4 * (i + 1),
                                        128 * q:128 * (q + 1)],
                                    start=True, stop=True,
                                )
                            psx = ps[:].rearrange("p (f x) -> p x f", x=128)
                            nc.vector.tensor_sub(
                                outq3[:, :, 16 * (b + 1):16 * (b + 2)], fmq, psx)
                        nc.sync.dma_start(
                            out=out_d[128 * t:128 * (t + 1),
                                      128 * q:128 * (q + 1), :],
                            in_=outq3,
                        )
    if compile:
        nc.compile()
    return nc


def _get_program():
    if "nc" not in _cache:
        _cache["nc"] = _build_program()
    return _cache["nc"]


def kernel(feature_map, kernel, bias):
    from concourse.bass_utils import run_bass_kernel_spmd

    feature_map = np.ascontiguousarray(feature_map, dtype=np.float32)
    kernel = np.ascontiguousarray(kernel, dtype=np.float32)
    bias = np.ascontiguousarray(bias, dtype=np.float32)
    B = feature_map.shape[0]
    assert B == 8

    poolw, hup, kt, bias2, _, _ = _host_consts(kernel, bias)
    nc = _get_program()
    in_maps = [
        {
            "feature_map": feature_map[b],
            "poolw": poolw,
            "hup": hup,
            "ktaps": kt,
            "bias2": bias2,
        }
        for b in range(B)
    ]
    res = run_bass_kernel_spmd(nc, in_maps, list(range(B)))
    out = np.stack([res.results[b]["out"] for b in range(B)])
    return out
